# revision 20
# baseline (speedup 1.0000x reference)
"""LGCN encoder kernel for 8 Trainium2 NeuronCores.

Computes out = 0.5*(ego + V @ (filt[:,None] * (V^T @ ego))) with
ego = concat(user_emb, item_emb), row-sharded over N across 8 cores.

All bulk operands travel as bf16 (PSUM accumulates fp32; ~5x under the
2e-2 error budget). v arrives in BOTH layouts, host-prepared: p-major
[P, T, F] tiles for pass 1 (5KB contiguous DMA descriptors) and
[F, rows] for pass 2.

Structure: pass-1 streams v row-tiles at full DMA rate (its tail gates
the collective). M = 0.5*filt*proj is formed LOCALLY pre-AllReduce
(scale + transpose are linear, so they commute with the reduction),
making the AllReduce payload the final [F/128, D] chunk stack and the
post-AllReduce critical path a single 64KB DMA. The pass-2 v^T copy is
held entirely in SBUF (~100KB/partition); its DMAs are gated by a
register dependency so the scheduler cannot interleave them with the
pass-1 stream.
"""

import sys

if "/opt/trn_rl_repo" not in sys.path:
    sys.path.insert(0, "/opt/trn_rl_repo")

import ml_dtypes
import numpy as np

from concourse import bacc, bass, mybir, tile
from concourse.bass_utils import run_bass_kernel_spmd

N_CORES = 8
USER_NUM = 50000
ITEM_NUM = 50000
N_FULL = USER_NUM + ITEM_NUM          # 100000
F = 512
D = 64
P = 128                               # partitions / n-tile rows
ROWS = 12800                          # rows per core, 100 tiles of 128
NPAD = ROWS * N_CORES                 # 102400
N_TILES = ROWS // P                   # 100
BLK = 512                             # pass-2 n-block (free dim)
N_BLKS = ROWS // BLK                  # 25
FC = F // P                           # 4 f-chunks of 128

F32 = mybir.dt.float32
BF16 = mybir.dt.bfloat16
I32 = mybir.dt.int32

# pass-1 n-tiles per v-row DMA; taper so the last loads (which gate
# the AllReduce) complete quickly
V_GROUPS = [5] * 18 + [4, 3, 2, 1]
# pass-2 v^T tiles: FC chunks x VT_J n-slices, all resident in SBUF
VT_J = 5
VT_L = ROWS // VT_J                   # 2560 (= 5 blocks of 512)


def _make_gate(nc, marker):
    """ScalarValue that is always true but carries a register dependency
    on `marker` (an int32 [1,1] SBUF tile): the cond folds into the DMA
    address computation, so gated DMAs cannot be scheduled before the
    marker is written."""
    r_raw = nc.sync.alloc_register()
    nc.sync.reg_load(r_raw, marker[0:1, 0:1])
    r_zero = nc.sync.alloc_register()
    nc.sync.reg_alu(r_zero, r_raw, 0, mybir.AluOpType.mult)
    return nc.sync.snap(r_zero, min_val=0, max_val=1) != 1


def _build(single_core=False):
    nc = bacc.Bacc(
        "TRN2",
        target_bir_lowering=False,
        debug=False,
        num_devices=1 if single_core else N_CORES,
    )
    # v_rows arrives host-shuffled p-major: vrp[p, t, f] = v[t*128+p, f]
    v_rows = nc.dram_tensor("v_rows", [P, N_TILES, F], BF16, kind="ExternalInput").ap()
    v_cols = nc.dram_tensor("v_cols", [F, ROWS], BF16, kind="ExternalInput").ap()
    # ego arrives host-shuffled to [p, t, d] (t = n-tile index, n = t*128+p).
    # egoh = 0.5*ego^T in bf16 feeds the transposed epilogue (ego is O(1)
    # vs filtered O(1e3)). fb = 0.5*filt broadcast to [D, F]. out is
    # stored transposed [d, n]; the host transposes it back.
    ego = nc.dram_tensor("ego", [P, N_TILES * D], BF16, kind="ExternalInput").ap()
    egoh = nc.dram_tensor("egoh", [D, ROWS], BF16, kind="ExternalInput").ap()
    fb = nc.dram_tensor("fb", [D, F], F32, kind="ExternalInput").ap()
    ident = nc.dram_tensor("ident", [D, D], BF16, kind="ExternalInput").ap()
    out = nc.dram_tensor("out", [D, ROWS], BF16, kind="ExternalOutput").ap()

    with tile.TileContext(nc) as tc:
        with (
            tc.tile_pool(name="const", bufs=1) as const_pool,
            tc.tile_pool(name="stream", bufs=8) as stream_pool,
            tc.tile_pool(name="small", bufs=1) as small_pool,
            tc.tile_pool(name="outp", bufs=4) as out_pool,
            tc.tile_pool(name="ps_proj", bufs=1, space="PSUM") as ps_proj,
            tc.tile_pool(name="ps_ft", bufs=3, space="PSUM") as ps_ft,
            tc.tile_pool(name="ps_tr", bufs=4, space="PSUM") as ps_tr,
            tc.tile_pool(name="dram", bufs=2, space="DRAM") as dram_pool,
        ):
            # consts go on the scalar ring so the sync ring starts with the
            # v stream immediately; ego gates the first matmul, split wide
            ego_all = const_pool.tile([P, N_TILES, D], BF16, tag="ego_all")
            ego_r = ego.rearrange("p (t d) -> p t d", d=D)
            for q in range(8):
                t0, t1 = q * 13, min((q + 1) * 13, N_TILES)
                nc.scalar.dma_start(
                    out=ego_all[:, t0:t1, :], in_=ego_r[:, t0:t1, :]
                )
            fb_sb = const_pool.tile([D, F], F32, tag="fb")
            nc.scalar.dma_start(out=fb_sb[:], in_=fb[:])
            ident_sb = const_pool.tile([D, D], BF16, tag="ident")
            nc.scalar.dma_start(out=ident_sb[:], in_=ident[:])

            # ---- pass 1: projT[d, f] += sum_n ego[n, d] * v[n, f].
            # Each arriving tile is also transposed SBUF->SBUF via the
            # DMA XBAR into the pass-2 [f, n] layout, so v is read from
            # HBM exactly once ----
            vt_all = const_pool.tile([P, FC, ROWS], BF16, tag="vt_all")
            projT_ps = ps_proj.tile([D, F], F32, tag="projT")
            t0 = 0
            for vg in V_GROUPS:
                v_g = stream_pool.tile([P, vg, F], BF16, tag="strm")
                nc.sync.dma_start(out=v_g[:], in_=v_rows[:, t0 : t0 + vg, :])
                for j in range(vg):
                    t = t0 + j
                    nc.tensor.matmul(
                        projT_ps[:],
                        lhsT=ego_all[:, t, :],
                        rhs=v_g[:, j, :],
                        start=(t == 0),
                        stop=(t == N_TILES - 1),
                    )
                    for c in range(FC):
                        nc.sync.dma_start_transpose(
                            out=vt_all[:, c, t * P : (t + 1) * P],
                            in_=v_g[:, j, c * P : (c + 1) * P],
                        )
                t0 += vg

            # ---- local M^T = 0.5*filt*proj partial, transposed to [f, d]
            # chunks BEFORE the AllReduce (linear ops commute with the
            # reduction), so post-AllReduce is a single DMA ----
            mT_sb = small_pool.tile([D, F], BF16, tag="mT")
            nc.vector.tensor_tensor(
                out=mT_sb[:], in0=projT_ps[:], in1=fb_sb[:],
                op=mybir.AluOpType.mult,
            )
            gate1_mk = small_pool.tile([1, 1], I32, tag="gate1_mk")
            nc.vector.tensor_copy(gate1_mk[:], projT_ps[0:1, 0:1])

            m_cat = small_pool.tile([P, FC, D], BF16, tag="m_cat")
            for c in range(FC):
                tr_ps = ps_tr.tile([P, D], BF16, tag="tr")
                nc.tensor.transpose(
                    tr_ps[:], mT_sb[:, c * P : (c + 1) * P], ident_sb[:]
                )
                nc.vector.tensor_copy(m_cat[:, c, :], tr_ps[:])

            # ---- AllReduce the local [F, D] M partial (bf16) ----
            ar_in = dram_pool.tile([P, FC * D], BF16, tag="ar_in")
            ar_out = dram_pool.tile([P, FC * D], BF16, tag="ar_out")
            nc.scalar.dma_start(
                out=ar_in[:], in_=m_cat.rearrange("p c d -> p (c d)")
            )
            if single_core:
                nc.scalar.dma_start(out=ar_out[:], in_=ar_in[:])
            else:
                nc.gpsimd.collective_compute(
                    "AllReduce",
                    mybir.AluOpType.add,
                    replica_groups=[list(range(N_CORES))],
                    ins=[ar_in.opt()],
                    outs=[ar_out.opt()],
                )
            m_all = small_pool.tile([P, FC, D], BF16, tag="m_all")
            nc.scalar.dma_start(
                out=m_all.rearrange("p c d -> p (c d)"), in_=ar_out[:]
            )

            # epilogue operand: gated behind the pass-1 stream
            gate1 = _make_gate(nc, gate1_mk)
            egoh_sb = const_pool.tile([D, ROWS], BF16, tag="egoh")
            nc.sync.dma_start(
                out=egoh_sb[:], in_=egoh[:], cond=gate1, cond_hint=True
            )

            # ---- pass 2: outT[d, n] = sum_f M[f, d]*vT[f, n] + 0.5*egoT,
            # epilogue add fused into the PSUM drain, bf16 store ----
            for b in range(N_BLKS):
                ftT_ps = ps_ft.tile([D, BLK], F32, tag="ftT")
                for c in range(FC):
                    nc.tensor.matmul(
                        ftT_ps[:],
                        lhsT=m_all[:, c, :],
                        rhs=vt_all[:, c, b * BLK : (b + 1) * BLK],
                        start=(c == 0),
                        stop=(c == FC - 1),
                    )
                out_blk = out_pool.tile([D, BLK], BF16, tag="o")
                nc.vector.tensor_add(
                    out_blk[:],
                    ftT_ps[:],
                    egoh_sb[:, b * BLK : (b + 1) * BLK],
                )
                nc.scalar.dma_start(
                    out=out[:, b * BLK : (b + 1) * BLK], in_=out_blk[:]
                )

    nc.compile()
    return nc


_NC = {}


def _get_nc():
    if "nc" not in _NC:
        _NC["nc"] = _build()
    return _NC["nc"]


def _prep_in_maps(user_emb, item_emb, v, filt):
    bf = ml_dtypes.bfloat16
    ego = np.concatenate(
        [np.asarray(user_emb, np.float32), np.asarray(item_emb, np.float32)], axis=0
    )
    v = np.asarray(v, np.float32)
    filt = np.asarray(filt, np.float32)
    ego_pad = np.zeros((NPAD, D), np.float32)
    ego_pad[:N_FULL] = ego
    v_pad = np.zeros((NPAD, F), np.float32)
    v_pad[:N_FULL] = v
    ident = np.eye(D, dtype=bf)
    fb = np.ascontiguousarray(
        np.broadcast_to(0.5 * filt[None, :], (D, F)), dtype=np.float32
    )
    in_maps = []
    for c in range(N_CORES):
        sl = slice(c * ROWS, (c + 1) * ROWS)
        vr = v_pad[sl].astype(bf)                      # [12800, 512] bf16
        vrp = np.ascontiguousarray(                    # p-major [128, 100, 512]
            vr.reshape(N_TILES, P, F).transpose(1, 0, 2)
        )
        ego_shuf = np.ascontiguousarray(
            ego_pad[sl]
            .reshape(N_TILES, P, D)
            .transpose(1, 0, 2)
            .reshape(P, N_TILES * D)
        ).astype(bf)
        egoh = np.ascontiguousarray(
            (0.5 * ego_pad[sl].T).astype(bf)
        )
        in_maps.append(
            {
                "v_rows": vrp,
                "v_cols": np.ascontiguousarray(vr.T),
                "ego": ego_shuf,
                "egoh": egoh,
                "fb": fb,
                "ident": ident,
            }
        )
    return in_maps


def run(user_emb, item_emb, v, filt, trace=False, **trace_kwargs):
    nc = _get_nc()
    in_maps = _prep_in_maps(user_emb, item_emb, v, filt)
    res = run_bass_kernel_spmd(
        nc, in_maps, list(range(N_CORES)), trace=trace, **trace_kwargs
    )
    out = np.concatenate(
        [np.asarray(res.results[c]["out"]).T for c in range(N_CORES)], axis=0
    )[:N_FULL]
    return (out[:USER_NUM], out[USER_NUM:]), res


def kernel(user_emb, item_emb, v, filt, k=None, **_unused):
    (user_out, item_out), _ = run(user_emb, item_emb, v, filt)
    return (
        np.asarray(user_out, np.float32),
        np.asarray(item_out, np.float32),
    )


# revision 21
# speedup vs baseline: 4.4837x; 4.4837x over previous
"""LGCN encoder kernel for 8 Trainium2 NeuronCores.

Computes out = 0.5*(ego + V @ (filt[:,None] * (V^T @ ego))) with
ego = concat(user_emb, item_emb), row-sharded over N across 8 cores.

All bulk operands travel as bf16 (PSUM accumulates fp32; ~5x under the
2e-2 error budget). v arrives in BOTH layouts, host-prepared: p-major
[P, T, F] tiles for pass 1 (5KB contiguous DMA descriptors) and
[F, rows] for pass 2.

Structure: pass-1 streams v row-tiles at full DMA rate (its tail gates
the collective). M = 0.5*filt*proj is formed LOCALLY pre-AllReduce
(scale + transpose are linear, so they commute with the reduction),
making the AllReduce payload the final [F/128, D] chunk stack and the
post-AllReduce critical path a single 64KB DMA. The pass-2 v^T copy is
held entirely in SBUF (~100KB/partition); its DMAs are gated by a
register dependency so the scheduler cannot interleave them with the
pass-1 stream.
"""

import sys

if "/opt/trn_rl_repo" not in sys.path:
    sys.path.insert(0, "/opt/trn_rl_repo")

import ml_dtypes
import numpy as np

from concourse import bacc, bass, mybir, tile
from concourse.bass_utils import run_bass_kernel_spmd

N_CORES = 8
USER_NUM = 50000
ITEM_NUM = 50000
N_FULL = USER_NUM + ITEM_NUM          # 100000
F = 512
D = 64
P = 128                               # partitions / n-tile rows
ROWS = 12800                          # rows per core, 100 tiles of 128
NPAD = ROWS * N_CORES                 # 102400
N_TILES = ROWS // P                   # 100
BLK = 512                             # pass-2 n-block (free dim)
N_BLKS = ROWS // BLK                  # 25
FC = F // P                           # 4 f-chunks of 128

F32 = mybir.dt.float32
BF16 = mybir.dt.bfloat16
I32 = mybir.dt.int32

# pass-1 n-tiles per v-row DMA; taper so the last loads (which gate
# the AllReduce) complete quickly
V_GROUPS = [5] * 18 + [4, 3, 2, 1]
# pass-2 v^T tiles: FC chunks x VT_J n-slices, all resident in SBUF
VT_J = 5
VT_L = ROWS // VT_J                   # 2560 (= 5 blocks of 512)


def _make_gate(nc, marker):
    """ScalarValue that is always true but carries a register dependency
    on `marker` (an int32 [1,1] SBUF tile): the cond folds into the DMA
    address computation, so gated DMAs cannot be scheduled before the
    marker is written."""
    r_raw = nc.sync.alloc_register()
    nc.sync.reg_load(r_raw, marker[0:1, 0:1])
    r_zero = nc.sync.alloc_register()
    nc.sync.reg_alu(r_zero, r_raw, 0, mybir.AluOpType.mult)
    return nc.sync.snap(r_zero, min_val=0, max_val=1) != 1


def _build(single_core=False):
    nc = bacc.Bacc(
        "TRN2",
        target_bir_lowering=False,
        debug=False,
        num_devices=1 if single_core else N_CORES,
    )
    # v_rows arrives host-shuffled p-major: vrp[p, t, f] = v[t*128+p, f]
    v_rows = nc.dram_tensor("v_rows", [P, N_TILES, F], BF16, kind="ExternalInput").ap()
    v_cols = nc.dram_tensor("v_cols", [F, ROWS], BF16, kind="ExternalInput").ap()
    # ego arrives host-shuffled to [p, t, d] (t = n-tile index, n = t*128+p).
    # egoh = 0.5*ego^T in bf16 feeds the transposed epilogue (ego is O(1)
    # vs filtered O(1e3)). fb = 0.5*filt broadcast to [D, F]. out is
    # stored transposed [d, n]; the host transposes it back.
    ego = nc.dram_tensor("ego", [P, N_TILES * D], BF16, kind="ExternalInput").ap()
    egoh = nc.dram_tensor("egoh", [D, ROWS], BF16, kind="ExternalInput").ap()
    fb = nc.dram_tensor("fb", [D, F], F32, kind="ExternalInput").ap()
    ident = nc.dram_tensor("ident", [D, D], BF16, kind="ExternalInput").ap()
    out = nc.dram_tensor("out", [D, ROWS], BF16, kind="ExternalOutput").ap()

    with tile.TileContext(nc) as tc:
        with (
            tc.tile_pool(name="const", bufs=1) as const_pool,
            tc.tile_pool(name="stream", bufs=8) as stream_pool,
            tc.tile_pool(name="small", bufs=1) as small_pool,
            tc.tile_pool(name="outp", bufs=4) as out_pool,
            tc.tile_pool(name="ps_proj", bufs=1, space="PSUM") as ps_proj,
            tc.tile_pool(name="ps_ft", bufs=3, space="PSUM") as ps_ft,
            tc.tile_pool(name="ps_tr", bufs=4, space="PSUM") as ps_tr,
            tc.tile_pool(name="dram", bufs=2, space="DRAM") as dram_pool,
        ):
            # consts go on the scalar ring so the sync ring starts with the
            # v stream immediately; ego gates the first matmul, split wide
            ego_all = const_pool.tile([P, N_TILES, D], BF16, tag="ego_all")
            ego_r = ego.rearrange("p (t d) -> p t d", d=D)
            for q in range(8):
                t0, t1 = q * 13, min((q + 1) * 13, N_TILES)
                nc.scalar.dma_start(
                    out=ego_all[:, t0:t1, :], in_=ego_r[:, t0:t1, :]
                )
            fb_sb = const_pool.tile([D, F], F32, tag="fb")
            nc.scalar.dma_start(out=fb_sb[:], in_=fb[:])
            ident_sb = const_pool.tile([D, D], BF16, tag="ident")
            nc.scalar.dma_start(out=ident_sb[:], in_=ident[:])

            # ---- pass 1: projT[d, f] += sum_n ego[n, d] * v[n, f] ----
            projT_ps = ps_proj.tile([D, F], F32, tag="projT")
            t0 = 0
            for vg in V_GROUPS:
                v_g = stream_pool.tile([P, vg, F], BF16, tag="strm")
                nc.sync.dma_start(out=v_g[:], in_=v_rows[:, t0 : t0 + vg, :])
                for j in range(vg):
                    t = t0 + j
                    nc.tensor.matmul(
                        projT_ps[:],
                        lhsT=ego_all[:, t, :],
                        rhs=v_g[:, j, :],
                        start=(t == 0),
                        stop=(t == N_TILES - 1),
                    )
                t0 += vg

            # ---- local M^T = 0.5*filt*proj partial, transposed to [f, d]
            # chunks BEFORE the AllReduce (linear ops commute with the
            # reduction), so post-AllReduce is a single DMA ----
            mT_sb = small_pool.tile([D, F], BF16, tag="mT")
            nc.vector.tensor_tensor(
                out=mT_sb[:], in0=projT_ps[:], in1=fb_sb[:],
                op=mybir.AluOpType.mult,
            )
            gate1_mk = small_pool.tile([1, 1], I32, tag="gate1_mk")
            nc.vector.tensor_copy(gate1_mk[:], projT_ps[0:1, 0:1])

            m_cat = small_pool.tile([P, FC, D], BF16, tag="m_cat")
            for c in range(FC):
                tr_ps = ps_tr.tile([P, D], BF16, tag="tr")
                nc.tensor.transpose(
                    tr_ps[:], mT_sb[:, c * P : (c + 1) * P], ident_sb[:]
                )
                nc.vector.tensor_copy(m_cat[:, c, :], tr_ps[:])

            # ---- AllReduce the local [F, D] M partial (bf16) ----
            ar_in = dram_pool.tile([P, FC * D], BF16, tag="ar_in")
            ar_out = dram_pool.tile([P, FC * D], BF16, tag="ar_out")
            nc.scalar.dma_start(
                out=ar_in[:], in_=m_cat.rearrange("p c d -> p (c d)")
            )
            if single_core:
                nc.scalar.dma_start(out=ar_out[:], in_=ar_in[:])
            else:
                nc.gpsimd.collective_compute(
                    "AllReduce",
                    mybir.AluOpType.add,
                    replica_groups=[list(range(N_CORES))],
                    ins=[ar_in.opt()],
                    outs=[ar_out.opt()],
                )
            m_all = small_pool.tile([P, FC, D], BF16, tag="m_all")
            nc.scalar.dma_start(
                out=m_all.rearrange("p c d -> p (c d)"), in_=ar_out[:]
            )

            # ---- pass-2 v^T loads: gated by a register dependency so the
            # scheduler cannot interleave them with the pass-1 stream
            # (whose tail gates the collective). Few, large DMAs: each
            # gated trigger costs ~1.1us of serial sync-engine register
            # evaluation, so 6 triggers instead of 24. The 13MB land
            # during the ~20us collective-entry window, before the mesh's
            # data phase (which concurrent DMA would stretch ~2x).
            gate1 = _make_gate(nc, gate1_mk)

            # epilogue operand first: it must not sit behind the v^T bulk
            egoh_sb = const_pool.tile([D, ROWS], BF16, tag="egoh")
            nc.sync.dma_start(
                out=egoh_sb[:], in_=egoh[:], cond=gate1, cond_hint=True
            )

            vt_all = const_pool.tile([P, FC, ROWS], BF16, tag="vt_all")
            vc_r = v_cols.rearrange("(c p) n -> p c n", p=P)
            for jj in range(VT_J):
                n0, n1 = jj * VT_L, (jj + 1) * VT_L
                nc.sync.dma_start(
                    out=vt_all[:, :, n0:n1],
                    in_=vc_r[:, :, n0:n1],
                    cond=gate1,
                    cond_hint=True,
                )

            # ---- pass 2: outT[d, n] = sum_f M[f, d]*vT[f, n] + 0.5*egoT,
            # epilogue add fused into the PSUM drain, bf16 store ----
            for b in range(N_BLKS):
                ftT_ps = ps_ft.tile([D, BLK], F32, tag="ftT")
                for c in range(FC):
                    nc.tensor.matmul(
                        ftT_ps[:],
                        lhsT=m_all[:, c, :],
                        rhs=vt_all[:, c, b * BLK : (b + 1) * BLK],
                        start=(c == 0),
                        stop=(c == FC - 1),
                    )
                out_blk = out_pool.tile([D, BLK], BF16, tag="o")
                nc.vector.tensor_add(
                    out_blk[:],
                    ftT_ps[:],
                    egoh_sb[:, b * BLK : (b + 1) * BLK],
                )
                nc.scalar.dma_start(
                    out=out[:, b * BLK : (b + 1) * BLK], in_=out_blk[:]
                )

    nc.compile()
    return nc


_NC = {}


def _get_nc():
    if "nc" not in _NC:
        _NC["nc"] = _build()
    return _NC["nc"]


def _prep_in_maps(user_emb, item_emb, v, filt):
    bf = ml_dtypes.bfloat16
    ego = np.concatenate(
        [np.asarray(user_emb, np.float32), np.asarray(item_emb, np.float32)], axis=0
    )
    v = np.asarray(v, np.float32)
    filt = np.asarray(filt, np.float32)
    ego_pad = np.zeros((NPAD, D), np.float32)
    ego_pad[:N_FULL] = ego
    v_pad = np.zeros((NPAD, F), np.float32)
    v_pad[:N_FULL] = v
    ident = np.eye(D, dtype=bf)
    fb = np.ascontiguousarray(
        np.broadcast_to(0.5 * filt[None, :], (D, F)), dtype=np.float32
    )
    in_maps = []
    for c in range(N_CORES):
        sl = slice(c * ROWS, (c + 1) * ROWS)
        vr = v_pad[sl].astype(bf)                      # [12800, 512] bf16
        vrp = np.ascontiguousarray(                    # p-major [128, 100, 512]
            vr.reshape(N_TILES, P, F).transpose(1, 0, 2)
        )
        ego_shuf = np.ascontiguousarray(
            ego_pad[sl]
            .reshape(N_TILES, P, D)
            .transpose(1, 0, 2)
            .reshape(P, N_TILES * D)
        ).astype(bf)
        egoh = np.ascontiguousarray(
            (0.5 * ego_pad[sl].T).astype(bf)
        )
        in_maps.append(
            {
                "v_rows": vrp,
                "v_cols": np.ascontiguousarray(vr.T),
                "ego": ego_shuf,
                "egoh": egoh,
                "fb": fb,
                "ident": ident,
            }
        )
    return in_maps


def run(user_emb, item_emb, v, filt, trace=False, **trace_kwargs):
    nc = _get_nc()
    in_maps = _prep_in_maps(user_emb, item_emb, v, filt)
    res = run_bass_kernel_spmd(
        nc, in_maps, list(range(N_CORES)), trace=trace, **trace_kwargs
    )
    out = np.concatenate(
        [np.asarray(res.results[c]["out"]).T for c in range(N_CORES)], axis=0
    )[:N_FULL]
    return (out[:USER_NUM], out[USER_NUM:]), res


def kernel(user_emb, item_emb, v, filt, k=None, **_unused):
    (user_out, item_out), _ = run(user_emb, item_emb, v, filt)
    return (
        np.asarray(user_out, np.float32),
        np.asarray(item_out, np.float32),
    )
